# revision 7
# baseline (speedup 1.0000x reference)
"""Trainium2 Bass kernel for nn_GCN1PoolNorm: 3-layer GCN + shared BatchNorm +
global max pool + MLP head, SPMD across 8 NeuronCores.

Self-contained: hardcodes the problem dims; takes FULL inputs, returns FULL
output [N_GRAPHS, N_CLASSES].

Design (per core = one 1/8 dst-shard of nodes):
- Shared bf16 node table in DRAM, rows padded to 256B ([N+1, 128] bf16,
  feats in cols 0:64). Table rows hold h_tilde = act * dis (dis = rsqrt(deg+1)).
- Message aggregation: extended dma_gather (int16 idx, split at row 32768)
  pulls edge source rows; PE matmul per 128-edge chunk against a resident fp8
  one-hot (edge -> dst slot) accumulates PSUM [64 feat, <=128 dst] per node
  tile (feat-major). Self-loops are extra self-edges.
- U.T = psum * dis_rep; Z.T = W.T @ U.T (PE, lhsT=W); BN stats via ACT
  accum_out; stats AllReduce; BN affine + relu fused in one ACT op
  (per-partition scale/bias = per-feature in feat-major); * dis; PE transpose
  back to node-major; DMA to table shard; AllGather publishes the table.
- Pooling: graphs align exactly to cores (8/core); free-axis reduce_max
  segments; MLP head feat-major; out [8, 10] per core, host concatenates.
"""
import numpy as np
import ml_dtypes

from concourse import bass, bacc, mybir, tile
from concourse.bass import _add_dep_helper
from concourse.bass_utils import run_bass_kernel_spmd
from concourse.masks import make_identity

f32 = mybir.dt.float32
bf16 = mybir.dt.bfloat16
i16 = mybir.dt.int16
fp8 = mybir.dt.float8e4

N_CORES = 8
P = 128          # partition / chunk quantum
ROW = 128        # padded table row elems (bf16 -> 256B)
D = 64           # feature dim
SPLIT = 32768    # int16 index range per table half
BN_EPS = 1e-5
MAX_IDX_PER_CALL = 8192  # HW-validated dma_gather limit (single_packet=False)
GROUP_TILES = 3  # node tiles per gather group


# ---------------------------------------------------------------- host prep

def _prep(x, edge_index, batch, n_classes):
    """Build per-core shard data + the shared static layout (identical across
    cores; per-tile chunk counts are maxed over cores)."""
    n_nodes = x.shape[0]
    n_graphs = int(batch.max()) + 1
    assert n_nodes % N_CORES == 0
    nsh = n_nodes // N_CORES                    # nodes per core
    ntile = (nsh + P - 1) // P                  # node tiles per core
    tsz = [min(P, nsh - t * P) for t in range(ntile)]

    src = edge_index[0].astype(np.int64)
    dst = edge_index[1].astype(np.int64)
    deg = np.bincount(dst, minlength=n_nodes).astype(np.int64)

    # per (core, tile): edge lists split by src half, self-edges appended
    # global tile id = core*ntile + local tile (nsh need not be 128-aligned)
    tile_raw = (dst // nsh) * ntile + (dst % nsh) // P
    order = np.argsort(tile_raw, kind="stable")
    src_s, dst_s, tile_of = src[order], dst[order], tile_raw[order]
    bounds = np.searchsorted(tile_of, np.arange(N_CORES * ntile + 1))

    S_lo = np.zeros(ntile, np.int64)
    S_hi = np.zeros(ntile, np.int64)
    per_ct = {}
    for c in range(N_CORES):
        for t in range(ntile):
            g = c * ntile + t
            es, ee = bounds[g], bounds[g + 1]
            s_seg, d_seg = src_s[es:ee], dst_s[es:ee]
            base = c * nsh + t * P
            selfn = np.arange(base, base + tsz[t], dtype=np.int64)
            allsrc = np.concatenate([s_seg, selfn])
            alldst = np.concatenate([d_seg, selfn]) - base   # local slot 0..tsz
            lo_m = allsrc < SPLIT
            per_ct[(c, t)] = (allsrc[lo_m], alldst[lo_m],
                              allsrc[~lo_m] - SPLIT, alldst[~lo_m])
            S_lo[t] = max(S_lo[t], (lo_m.sum() + P - 1) // P)
            S_hi[t] = max(S_hi[t], ((~lo_m).sum() + P - 1) // P)

    # gather groups of GROUP_TILES tiles; per group slots = [lo(t..) | hi(t..)]
    groups = []
    for t0 in range(0, ntile, GROUP_TILES):
        ts = list(range(t0, min(t0 + GROUP_TILES, ntile)))
        glo = int(sum(S_lo[t] for t in ts))
        ghi = int(sum(S_hi[t] for t in ts))
        assert glo * P <= MAX_IDX_PER_CALL and ghi * P <= MAX_IDX_PER_CALL, \
            (glo, ghi)
        groups.append((ts, glo, ghi))
    sgmax = max(g[1] + g[2] for g in groups)
    nchunk = int(sum(g[1] + g[2] for g in groups))

    # per-core flat idx array (slot order) + onehot
    idx_flat = np.zeros((N_CORES, nchunk * P), np.int16)
    onehot = np.zeros((N_CORES, P, nchunk, P), np.float32)
    # chunk ranges per (group, tile): (lo_start, lo_cnt, hi_start, hi_cnt)
    chunk_map = []
    coff = 0
    for (ts, glo, ghi) in groups:
        lo_ofs, hi_ofs = coff, coff + glo
        for t in ts:
            chunk_map.append((t, lo_ofs, int(S_lo[t]), hi_ofs, int(S_hi[t])))
            for c in range(N_CORES):
                slo, dlo, shi, dhi = per_ct[(c, t)]
                # pad slots point at row 0 of each half: real data, finite;
                # their onehot columns are zero so they contribute nothing.
                for (seg_s, seg_d, ofs, scnt, half_pad) in (
                        (slo, dlo, lo_ofs, int(S_lo[t]), 0),
                        (shi, dhi, hi_ofs, int(S_hi[t]), 0)):
                    n = len(seg_s)
                    padded = scnt * P
                    buf = np.full(padded, half_pad, np.int64)
                    buf[:n] = seg_s
                    idx_flat[c, ofs * P: ofs * P + padded] = buf.astype(np.int16)
                    lanes = np.arange(padded) % P
                    chunks = ofs + np.arange(padded) // P
                    onehot[c, lanes[:n], chunks[:n], seg_d] = 1.0
            lo_ofs += int(S_lo[t])
            hi_ofs += int(S_hi[t])
        coff += glo + ghi
    assert coff == nchunk

    # wrapped idx per gather call: call block cols = S*8 int16
    idxw_cols = nchunk * 8
    idx_w = np.zeros((N_CORES, P, idxw_cols), np.int16)
    call_blocks = []  # (slot_ofs, nslots) per call, in group order lo,hi
    coff = 0
    for (ts, glo, ghi) in groups:
        for nsl in (glo, ghi):
            if nsl:
                call_blocks.append((coff, nsl))
            coff += nsl
    for c in range(N_CORES):
        for (ofs, nsl) in call_blocks:
            flat = idx_flat[c, ofs * P:(ofs + nsl) * P]
            w = np.tile(flat.reshape(-1, 16).T, (8, 1))   # [128, nsl*8]
            idx_w[c, :, ofs * 8:(ofs + nsl) * 8] = w

    # deg layouts (fp32)
    deg_pt = np.zeros((N_CORES, P, ntile), np.float32)
    deg_row = np.zeros((N_CORES, 1, nsh), np.float32)
    for c in range(N_CORES):
        dsh = deg[c * nsh:(c + 1) * nsh].astype(np.float32)
        deg_row[c, 0, :] = dsh
        for t in range(ntile):
            deg_pt[c, :tsz[t], t] = dsh[t * P:t * P + tsz[t]]

    # pooling segments: per tile list of (col0, col1, local graph id)
    gb = np.searchsorted(batch, np.arange(n_graphs + 1))
    gpc = n_graphs // N_CORES
    for c in range(N_CORES):
        assert gb[c * gpc] == c * nsh, "graphs must align to core shards"
    loc0 = gb[:gpc + 1] - 0  # core-0 local boundaries
    for c in range(N_CORES):
        locc = gb[c * gpc:(c + 1) * gpc + 1] - c * nsh
        assert np.array_equal(locc, loc0), "graph pattern must match across cores"
    pool_segs = []
    for t in range(ntile):
        a, b = t * P, t * P + tsz[t]
        for g in range(gpc):
            s, e = max(a, int(loc0[g])), min(b, int(loc0[g + 1]))
            if s < e:
                pool_segs.append((t, s - a, e - a, g, s == int(loc0[g])))

    cfg = dict(n_nodes=n_nodes, nsh=nsh, ntile=ntile, tsz=tsz,
               groups=groups, sgmax=sgmax, nchunk=nchunk,
               chunk_map=chunk_map, call_blocks=call_blocks,
               idxw_cols=idxw_cols, pool_segs=pool_segs, gpc=gpc,
               n_classes=n_classes, n_graphs=n_graphs)
    data = dict(idx_w=idx_w, onehot=onehot, deg_pt=deg_pt, deg_row=deg_row)
    return cfg, data


# ---------------------------------------------------------------- device build

def _build(cfg, reps=1):
    nsh, ntile, tsz = cfg["nsh"], cfg["ntile"], cfg["tsz"]
    nchunk, sgmax = cfg["nchunk"], cfg["sgmax"]
    ncls, gpc = cfg["n_classes"], cfg["gpc"]
    n_nodes = cfg["n_nodes"]
    nshp = ntile * P  # padded shard cols for feat-major buffers

    nc = bacc.Bacc(trn_type="TRN2", target_bir_lowering=False, debug=False,
                   num_devices=N_CORES)

    # inputs
    x_sh = nc.dram_tensor("x_sh", [nsh, D], f32, kind="ExternalInput").ap()
    idx_w = nc.dram_tensor("idx_w", [P, cfg["idxw_cols"]], i16,
                           kind="ExternalInput").ap()
    onehot = nc.dram_tensor("onehot", [P, nchunk, P], fp8,
                            kind="ExternalInput").ap()
    deg_pt = nc.dram_tensor("deg_pt", [P, ntile], f32, kind="ExternalInput").ap()
    deg_row = nc.dram_tensor("deg_row", [1, nsh], f32, kind="ExternalInput").ap()
    Ws = [nc.dram_tensor(f"W{i}", [D, D], bf16, kind="ExternalInput").ap()
          for i in (1, 2, 3)]
    gamma = nc.dram_tensor("gamma", [D, 1], f32, kind="ExternalInput").ap()
    beta = nc.dram_tensor("beta", [D, 1], f32, kind="ExternalInput").ap()
    lin1w = nc.dram_tensor("lin1w", [D, D], bf16, kind="ExternalInput").ap()
    lin1b = nc.dram_tensor("lin1b", [D, 1], f32, kind="ExternalInput").ap()
    lin2w = nc.dram_tensor("lin2w", [D, ncls], bf16, kind="ExternalInput").ap()
    lin2b = nc.dram_tensor("lin2b", [ncls, 1], f32, kind="ExternalInput").ap()
    out = nc.dram_tensor("out", [gpc, ncls], f32, kind="ExternalOutput").ap()

    # internal DRAM
    table = nc.dram_tensor("table", [n_nodes + P, ROW], bf16,
                           addr_space="Shared").ap()
    tshard = nc.dram_tensor("tshard", [nsh, ROW], bf16).ap()
    stats_in = nc.dram_tensor("stats_in", [D, 2], f32).ap()
    stats_out = nc.dram_tensor("stats_out", [D, 2], f32,
                               addr_space="Shared").ap()

    with tile.TileContext(nc) as tc:
        with (
            tc.tile_pool(name="const", bufs=1) as cpool,
            tc.tile_pool(name="gbuf", bufs=2) as gpool,
            tc.tile_pool(name="work", bufs=3) as wpool,
            tc.tile_pool(name="psacc", bufs=3, space="PSUM") as ps_acc,
            tc.tile_pool(name="psz", bufs=2, space="PSUM") as ps_z,
            tc.tile_pool(name="pstr", bufs=2, space="PSUM") as ps_tr,
        ):
            # ---- residents
            oh_sb = cpool.tile([P, nchunk, P], fp8)
            nc.sync.dma_start(out=oh_sb[:], in_=onehot[:])
            dis_pt = cpool.tile([P, ntile], f32)
            dis_rep = cpool.tile([D, nshp], bf16)
            zbuf = cpool.tile([D, nshp], bf16)
            act3 = zbuf  # last layer reuses the Z buffer (lifetimes disjoint)
            sums = cpool.tile([D, ntile], f32)
            sums2 = cpool.tile([D, ntile], f32)
            W_sb = [cpool.tile([D, D], bf16, tag=f"W{i}", name=f"W{i}_sb") for i in range(3)]
            for i in range(3):
                nc.sync.dma_start(out=W_sb[i][:], in_=Ws[i][:])
            gamma_sb = cpool.tile([D, 1], f32, tag="gamma")
            beta_sb = cpool.tile([D, 1], f32, tag="beta")
            nc.sync.dma_start(out=gamma_sb[:], in_=gamma[:])
            nc.sync.dma_start(out=beta_sb[:], in_=beta[:])
            l1w_sb = cpool.tile([D, D], bf16, tag="l1w")
            l1b_sb = cpool.tile([D, 1], f32, tag="l1b")
            l2w_sb = cpool.tile([D, ncls], bf16, tag="l2w")
            l2b_sb = cpool.tile([ncls, 1], f32, tag="l2b")
            nc.sync.dma_start(out=l1w_sb[:], in_=lin1w[:])
            nc.sync.dma_start(out=l1b_sb[:], in_=lin1b[:])
            nc.sync.dma_start(out=l2w_sb[:], in_=lin2w[:])
            nc.sync.dma_start(out=l2b_sb[:], in_=lin2b[:])
            ident = cpool.tile([P, P], bf16, tag="ident")
            make_identity(nc, ident[:])
            ones1 = cpool.tile([1, D], bf16, tag="ones1")
            nc.gpsimd.memset(ones1[:], 1.0)
            emb = cpool.tile([D, gpc], f32, tag="emb")
            eps_sb = cpool.tile([D, 1], f32, tag="eps")
            nc.gpsimd.memset(eps_sb[:], BN_EPS)

            # ---- dis: dis_pt = 1/sqrt(deg_pt + 1); dis_row likewise
            dptf = wpool.tile([P, ntile], f32, tag="dptf")
            nc.sync.dma_start(out=dptf[:], in_=deg_pt[:])
            nc.scalar.activation(dis_pt[:], dptf[:],
                                 mybir.ActivationFunctionType.Sqrt, bias=1.0)
            nc.vector.reciprocal(dis_pt[:], dis_pt[:])
            # dis_rep[f, n] = dis[n] broadcast across 64 partitions (PE K=1),
            # built from streamed [1, 512] slices of deg_row
            NB = 512
            for o in range(0, nsh, NB):
                w = min(NB, nsh - o)
                dsl = wpool.tile([1, NB], f32, tag="dsl")
                nc.sync.dma_start(out=dsl[:, :w], in_=deg_row[:, o:o + w])
                nc.scalar.activation(dsl[:, :w], dsl[:, :w],
                                     mybir.ActivationFunctionType.Sqrt, bias=1.0)
                nc.vector.reciprocal(dsl[:, :w], dsl[:, :w])
                dslb = wpool.tile([1, NB], bf16, tag="dslb")
                nc.vector.tensor_copy(dslb[:, :w], dsl[:, :w])
                pb = ps_z.tile([D, NB], f32, tag="zt", space="PSUM")
                nc.tensor.matmul(pb[:, :w], lhsT=ones1[:], rhs=dslb[:, :w],
                                 start=True, stop=True)
                nc.vector.tensor_copy(dis_rep[:, o:o + w], pb[:, :w])

            # ---- table0 = bf16(x * dis), node-major into tshard
            for t in range(ntile):
                w = tsz[t]
                xt = wpool.tile([P, D], f32, tag="xt")
                nc.sync.dma_start(out=xt[:w, :], in_=x_sh[t * P:t * P + w, :])
                xb = wpool.tile([P, D], bf16, tag="xb")
                nc.scalar.activation(xb[:w, :], xt[:w, :],
                                     mybir.ActivationFunctionType.Copy,
                                     scale=dis_pt[:w, t:t + 1])
                nc.sync.dma_start(out=tshard[t * P:t * P + w, :D], in_=xb[:w, :])
            nc.gpsimd.collective_compute(
                "AllGather", mybir.AluOpType.bypass,
                replica_groups=[list(range(N_CORES))],
                ins=[tshard[:, :].opt()], outs=[table[:n_nodes, :].opt()])

            # ---- layers
            for rep in range(reps):
                for li in range(3):
                    last = (li == 2)
                    Wl = W_sb[li]
                    for (ts, glo, ghi) in cfg["groups"]:
                        gb_sb = gpool.tile([P, sgmax, ROW], bf16, tag="G")
                        ix_sb = gpool.tile([P, sgmax * 8], i16, tag="ix")
                        cm = {t: cfg["chunk_map"][t] for t in ts}
                        grp_base = min(cm[t][1] for t in ts)
                        nc.sync.dma_start(
                            out=ix_sb[:, :(glo + ghi) * 8],
                            in_=idx_w[:, grp_base * 8:(grp_base + glo + ghi) * 8])
                        if glo:
                            nc.gpsimd.dma_gather(
                                out_ap=gb_sb[:, :glo, :],
                                in_ap=table[:min(SPLIT, n_nodes + P), :],
                                idxs_ap=ix_sb[:, :glo * 8],
                                num_idxs=glo * P, num_idxs_reg=glo * P,
                                elem_size=ROW, single_packet=False)
                        if ghi:
                            nc.gpsimd.dma_gather(
                                out_ap=gb_sb[:, glo:glo + ghi, :],
                                in_ap=table[SPLIT:, :],
                                idxs_ap=ix_sb[:, glo * 8:(glo + ghi) * 8],
                                num_idxs=ghi * P, num_idxs_reg=ghi * P,
                                elem_size=ROW, single_packet=False)

                        for t in ts:
                            w = tsz[t]
                            _, lo_ofs, lo_cnt, hi_ofs, hi_cnt = cm[t]
                            ps = ps_acc.tile([D, P], f32, tag="acc", space="PSUM")
                            ks = ([(lo_ofs + k) for k in range(lo_cnt)]
                                  + [(hi_ofs + k) for k in range(hi_cnt)])
                            for j, ck in enumerate(ks):
                                nc.tensor.matmul(
                                    ps[:, :w],
                                    lhsT=gb_sb[:, ck - grp_base, :D],
                                    rhs=oh_sb[:, ck, :w],
                                    start=(j == 0), stop=(j == len(ks) - 1))
                            # U.T = psum * dis (per-node, free axis)
                            u2t = wpool.tile([D, P], bf16, tag="u2t")
                            nc.vector.tensor_tensor(
                                out=u2t[:, :w], in0=ps[:, :w],
                                in1=dis_rep[:, t * P:t * P + w],
                                op=mybir.AluOpType.mult)
                            # Z.T = W.T @ U.T
                            psz = ps_z.tile([D, P], f32, tag="zt", space="PSUM")
                            nc.tensor.matmul(psz[:, :w], lhsT=Wl[:], rhs=u2t[:, :w],
                                             start=True, stop=True)
                            # stats + bf16 Z store (fused in ACT accum_out)
                            zslice = (act3 if last else zbuf)[:, t * P:t * P + w]
                            nc.scalar.activation(zslice, psz[:, :w],
                                                 mybir.ActivationFunctionType.Copy,
                                                 accum_out=sums[:, t:t + 1])
                            sq = wpool.tile([D, P], f32, tag="sq")
                            nc.scalar.activation(sq[:, :w], psz[:, :w],
                                                 mybir.ActivationFunctionType.Square,
                                                 accum_out=sums2[:, t:t + 1])

                    # ---- global BN stats
                    st = wpool.tile([D, 2], f32, tag="st")
                    nc.vector.reduce_sum(st[:, 0:1], sums[:],
                                         axis=mybir.AxisListType.X)
                    nc.vector.reduce_sum(st[:, 1:2], sums2[:],
                                         axis=mybir.AxisListType.X)
                    nc.sync.dma_start(out=stats_in[:], in_=st[:])
                    nc.gpsimd.collective_compute(
                        "AllReduce", mybir.AluOpType.add,
                        replica_groups=[list(range(N_CORES))],
                        ins=[stats_in[:, :].opt()], outs=[stats_out[:, :].opt()])
                    stg = wpool.tile([D, 2], f32, tag="stg")
                    nc.sync.dma_start(out=stg[:], in_=stats_out[:])
                    # mu = S/N ; var = S2/N - mu^2 ; s = gamma/sqrt(var+eps)
                    # tsh = beta - mu*s
                    mu = wpool.tile([D, 1], f32, tag="mu")
                    nc.scalar.activation(mu[:], stg[:, 0:1],
                                         mybir.ActivationFunctionType.Copy,
                                         scale=1.0 / n_nodes)
                    va = wpool.tile([D, 1], f32, tag="va")
                    nc.scalar.activation(va[:], stg[:, 1:2],
                                         mybir.ActivationFunctionType.Copy,
                                         scale=1.0 / n_nodes)
                    mu2 = wpool.tile([D, 1], f32, tag="mu2")
                    nc.vector.tensor_tensor(out=mu2[:], in0=mu[:], in1=mu[:],
                                            op=mybir.AluOpType.mult)
                    nc.vector.tensor_tensor(out=va[:], in0=va[:], in1=mu2[:],
                                            op=mybir.AluOpType.subtract)
                    nc.scalar.activation(va[:], va[:],
                                         mybir.ActivationFunctionType.Sqrt,
                                         bias=eps_sb[:])
                    nc.vector.reciprocal(va[:], va[:])
                    saff = wpool.tile([D, 1], f32, tag="saff")
                    nc.vector.tensor_tensor(out=saff[:], in0=gamma_sb[:], in1=va[:],
                                            op=mybir.AluOpType.mult)
                    tsh_ = wpool.tile([D, 1], f32, tag="tsh")
                    nc.vector.tensor_tensor(out=tsh_[:], in0=mu[:], in1=saff[:],
                                            op=mybir.AluOpType.mult)
                    nc.vector.tensor_tensor(out=tsh_[:], in0=beta_sb[:], in1=tsh_[:],
                                            op=mybir.AluOpType.subtract)

                    # ---- activation phase
                    for t in range(ntile):
                        w = tsz[t]
                        zsl = (act3 if last else zbuf)[:, t * P:t * P + w]
                        at = wpool.tile([D, P], bf16, tag="at")
                        nc.scalar.activation(at[:, :w], zsl,
                                             mybir.ActivationFunctionType.Relu,
                                             bias=tsh_[:], scale=saff[:])
                        if not last:
                            ht = wpool.tile([D, P], bf16, tag="ht")
                            nc.vector.tensor_tensor(
                                out=ht[:, :w], in0=at[:, :w],
                                in1=dis_rep[:, t * P:t * P + w],
                                op=mybir.AluOpType.mult)
                            ptr = ps_tr.tile([P, D], bf16, tag="tr", space="PSUM")
                            nc.tensor.transpose(ptr[:w, :], ht[:, :w],
                                                ident[:D, :D])
                            wr = wpool.tile([P, D], bf16, tag="wr")
                            nc.vector.tensor_copy(wr[:w, :], ptr[:w, :])
                            nc.sync.dma_start(
                                out=tshard[t * P:t * P + w, :D], in_=wr[:w, :])
                        else:
                            nc.vector.tensor_copy(act3[:, t * P:t * P + w],
                                                  at[:, :w])
                    if not last:
                        nc.gpsimd.collective_compute(
                            "AllGather", mybir.AluOpType.bypass,
                            replica_groups=[list(range(N_CORES))],
                            ins=[tshard[:, :].opt()],
                            outs=[table[:n_nodes, :].opt()])

            # ---- pooling (graph segments within tiles)
            first_seen = set()
            for (t, s0, s1, g, is_first) in cfg["pool_segs"]:
                tmp = wpool.tile([D, 1], f32, tag="ptmp")
                nc.vector.reduce_max(tmp[:], act3[:, t * P + s0:t * P + s1],
                                     axis=mybir.AxisListType.X)
                if g not in first_seen:
                    first_seen.add(g)
                    nc.vector.tensor_copy(emb[:, g:g + 1], tmp[:])
                else:
                    nc.vector.tensor_tensor(out=emb[:, g:g + 1],
                                            in0=emb[:, g:g + 1], in1=tmp[:],
                                            op=mybir.AluOpType.max)

            # ---- head
            emb_bf = wpool.tile([D, gpc], bf16, tag="embbf")
            nc.vector.tensor_copy(emb_bf[:], emb[:])
            ph = ps_z.tile([D, gpc], f32, tag="zt", space="PSUM")
            nc.tensor.matmul(ph[:], lhsT=l1w_sb[:], rhs=emb_bf[:],
                             start=True, stop=True)
            h1 = wpool.tile([D, gpc], bf16, tag="h1")
            nc.scalar.activation(h1[:], ph[:],
                                 mybir.ActivationFunctionType.Relu,
                                 bias=l1b_sb[:])
            po = ps_tr.tile([ncls, gpc], f32, tag="tr", space="PSUM")
            nc.tensor.matmul(po[:], lhsT=l2w_sb[:], rhs=h1[:],
                             start=True, stop=True)
            osb = wpool.tile([ncls, gpc], f32, tag="osb")
            nc.scalar.activation(osb[:], po[:],
                                 mybir.ActivationFunctionType.Identity,
                                 bias=l2b_sb[:])
            nc.sync.dma_start(out=out[:, :].rearrange("g c -> c g"), in_=osb[:])

    nc.compile()
    return nc


# ---------------------------------------------------------------- entry point

_CACHE = {}


def _get_built(cfg_key, cfg, reps):
    key = (cfg_key, reps)
    if key not in _CACHE:
        _CACHE[key] = _build(cfg, reps=reps)
    return _CACHE[key]


def kernel(x, edge_index, batch, W1, b1, W2, b2, W3, b3, gamma, beta,
           lin1_w, lin1_b, lin2_w, lin2_b, _reps=1, _return_nc=False):
    x = np.asarray(x, np.float32)
    edge_index = np.asarray(edge_index)
    batch = np.asarray(batch)
    n_nodes, d = x.shape
    ncls = np.asarray(lin2_w).shape[1]
    assert d == D

    cfg, data = _prep(x, edge_index, batch, ncls)
    nsh, gpc = cfg["nsh"], cfg["gpc"]

    # NOTE: b1/b2/b3 cancel inside BatchNorm (mean subtraction) - unused.
    W_bf = [np.asarray(w, np.float32).astype(ml_dtypes.bfloat16)
            for w in (W1, W2, W3)]
    in_maps = []
    for c in range(N_CORES):
        in_maps.append({
            "x_sh": x[c * nsh:(c + 1) * nsh].astype(np.float32),
            "idx_w": data["idx_w"][c],
            "onehot": data["onehot"][c].astype(ml_dtypes.float8_e4m3),
            "deg_pt": data["deg_pt"][c],
            "deg_row": data["deg_row"][c],
            "W1": W_bf[0], "W2": W_bf[1], "W3": W_bf[2],
            "gamma": np.asarray(gamma, np.float32).reshape(D, 1),
            "beta": np.asarray(beta, np.float32).reshape(D, 1),
            "lin1w": np.asarray(lin1_w, np.float32).astype(ml_dtypes.bfloat16),
            "lin1b": np.asarray(lin1_b, np.float32).reshape(D, 1),
            "lin2w": np.asarray(lin2_w, np.float32).astype(ml_dtypes.bfloat16),
            "lin2b": np.asarray(lin2_b, np.float32).reshape(ncls, 1),
        })

    cfg_key = (n_nodes, edge_index.shape[1], ncls)
    nc = _get_built(cfg_key, cfg, _reps)
    res = run_bass_kernel_spmd(nc, in_maps, core_ids=list(range(N_CORES)))
    outs = [res.results[c]["out"] for c in range(N_CORES)]
    full = np.concatenate(outs, axis=0).astype(np.float32)
    if _return_nc:
        return full, nc
    return full


# revision 11
# speedup vs baseline: 1.5322x; 1.5322x over previous
"""Trainium2 Bass kernel for nn_GCN1PoolNorm: 3-layer GCN + shared BatchNorm +
global max pool + MLP head, SPMD across 8 NeuronCores.

Self-contained: takes FULL inputs, returns FULL output [N_GRAPHS, N_CLASSES].

Design (per core = one 1/8 dst-shard of nodes) — DENSE aggregation:
- Message aggregation as a dense matmul: agg.T [64, nsh] = h_tilde.T @ A_c,
  where A_c [n_nodes, nsh] fp8 is the (0/1/multiplicity) adjacency column
  block for this core's dst shard, with self-loops on the diagonal. A_c is
  host-built once (graph is static), streamed from DRAM in bulk (fast HWDGE
  path), and reused by all 3 layers. This avoids the per-edge descriptor
  gather path entirely (~1us/descriptor on this runner's GPSIMD path).
- Table rows h_tilde = act * dis (dis = rsqrt(deg+1)) live in a Shared DRAM
  table [n_pad, 64] bf16, AllGather-published per layer; each layer loads the
  full table into SBUF as lhsT blocks [128, NB, 64].
- Per dst-range (512 cols): one PSUM bank accumulates NB chained matmuls
  (lhsT = table block bf16, rhs = A tile fp8). Downstream per node tile:
  U.T = psum * dis_rep; Z.T = W.T @ U.T; BN stats via ACT accum_out; stats
  AllReduce; BN affine+relu fused in one ACT op; * dis; PE transpose; DMA to
  table shard; AllGather.
- Pooling: graphs align exactly to cores; free-axis reduce_max segments;
  MLP head feat-major; out [gpc, 10] per core, host concatenates.
"""
import numpy as np
import ml_dtypes

from concourse import bacc, mybir, tile
from concourse.bass_utils import run_bass_kernel_spmd
from concourse.masks import make_identity

f32 = mybir.dt.float32
bf16 = mybir.dt.bfloat16
fp8 = mybir.dt.float8e4

N_CORES = 8
P = 128          # partition / block quantum
D = 64           # feature dim
RW = 512         # dst-range width (one PSUM bank)
BN_EPS = 1e-5


# ---------------------------------------------------------------- host prep

def _prep(x, edge_index, batch, n_classes):
    n_nodes = x.shape[0]
    n_graphs = int(batch.max()) + 1
    assert n_nodes % N_CORES == 0
    nsh = n_nodes // N_CORES                    # nodes per core
    ntile = (nsh + P - 1) // P                  # node tiles per core
    tsz = [min(P, nsh - t * P) for t in range(ntile)]
    NB = (n_nodes + P - 1) // P                 # src blocks (padded rows)
    npad = NB * P
    R = (nsh + RW - 1) // RW                    # dst ranges per core
    rsz = [min(RW, nsh - r * RW) for r in range(R)]

    src = edge_index[0].astype(np.int64)
    dst = edge_index[1].astype(np.int64)
    deg = np.bincount(dst, minlength=n_nodes).astype(np.int64)

    # A[c]: [R, NB, 128, RW] fp8; entry (r, b, s, d) = multiplicity of edge
    # (128b+s -> c*nsh + r*RW + d), plus self loops.
    A = []
    core_of = dst // nsh
    dloc = dst % nsh
    for c in range(N_CORES):
        m = core_of == c
        Ac = np.zeros((NB * P, nsh), np.uint8)
        np.add.at(Ac, (src[m], dloc[m]), 1)
        sn = np.arange(c * nsh, (c + 1) * nsh)
        Ac[sn, np.arange(nsh)] += 1             # self loops
        Af = np.zeros((R, NB, P, RW), ml_dtypes.float8_e4m3)
        for r in range(R):
            blk = Ac[:, r * RW:r * RW + rsz[r]].reshape(NB, P, rsz[r])
            Af[r, :, :, :rsz[r]] = blk.astype(ml_dtypes.float8_e4m3)
        A.append(Af)
        del Ac

    # deg layouts (fp32)
    deg_pt = np.zeros((N_CORES, P, ntile), np.float32)
    deg_row = np.zeros((N_CORES, 1, nsh), np.float32)
    for c in range(N_CORES):
        dsh = deg[c * nsh:(c + 1) * nsh].astype(np.float32)
        deg_row[c, 0, :] = dsh
        for t in range(ntile):
            deg_pt[c, :tsz[t], t] = dsh[t * P:t * P + tsz[t]]

    # pooling segments (identical across cores required for SPMD)
    gb = np.searchsorted(batch, np.arange(n_graphs + 1))
    gpc = n_graphs // N_CORES
    loc0 = gb[:gpc + 1].copy()
    for c in range(N_CORES):
        locc = gb[c * gpc:(c + 1) * gpc + 1] - c * nsh
        assert np.array_equal(locc, loc0), "graph pattern must match across cores"
    pool_segs = []
    for t in range(ntile):
        a, b = t * P, t * P + tsz[t]
        for g in range(gpc):
            s, e = max(a, int(loc0[g])), min(b, int(loc0[g + 1]))
            if s < e:
                pool_segs.append((t, s - a, e - a, g))

    cfg = dict(n_nodes=n_nodes, npad=npad, NB=NB, nsh=nsh, ntile=ntile,
               tsz=tsz, R=R, rsz=rsz, pool_segs=pool_segs, gpc=gpc,
               n_classes=n_classes, n_graphs=n_graphs)
    data = dict(A=A, deg_pt=deg_pt, deg_row=deg_row)
    return cfg, data


# ---------------------------------------------------------------- device build

def _build(cfg, reps=1):
    nsh, ntile, tsz = cfg["nsh"], cfg["ntile"], cfg["tsz"]
    NB, npad = cfg["NB"], cfg["npad"]
    R, rsz = cfg["R"], cfg["rsz"]
    ncls, gpc = cfg["n_classes"], cfg["gpc"]
    n_nodes = cfg["n_nodes"]
    nshp = ntile * P
    BC = 16                                     # A blocks per stream chunk

    nc = bacc.Bacc(trn_type="TRN2", target_bir_lowering=False, debug=False,
                   num_devices=N_CORES)

    x_sh = nc.dram_tensor("x_sh", [nsh, D], f32, kind="ExternalInput").ap()
    A_in = nc.dram_tensor("A", [R, NB, P, RW], fp8, kind="ExternalInput").ap()
    deg_pt = nc.dram_tensor("deg_pt", [P, ntile], f32, kind="ExternalInput").ap()
    deg_row = nc.dram_tensor("deg_row", [1, nsh], f32, kind="ExternalInput").ap()
    Ws = [nc.dram_tensor(f"W{i}", [D, D], bf16, kind="ExternalInput").ap()
          for i in (1, 2, 3)]
    gamma = nc.dram_tensor("gamma", [D, 1], f32, kind="ExternalInput").ap()
    beta = nc.dram_tensor("beta", [D, 1], f32, kind="ExternalInput").ap()
    lin1w = nc.dram_tensor("lin1w", [D, D], bf16, kind="ExternalInput").ap()
    lin1b = nc.dram_tensor("lin1b", [D, 1], f32, kind="ExternalInput").ap()
    lin2w = nc.dram_tensor("lin2w", [D, ncls], bf16, kind="ExternalInput").ap()
    lin2b = nc.dram_tensor("lin2b", [ncls, 1], f32, kind="ExternalInput").ap()
    out = nc.dram_tensor("out", [gpc, ncls], f32, kind="ExternalOutput").ap()

    table = nc.dram_tensor("table", [npad, D], bf16, addr_space="Shared").ap()
    tshard = nc.dram_tensor("tshard", [nsh, D], bf16).ap()
    stats_in = nc.dram_tensor("stats_in", [D, 2], f32).ap()
    stats_out = nc.dram_tensor("stats_out", [D, 2], f32,
                               addr_space="Shared").ap()

    with tile.TileContext(nc) as tc:
        with (
            tc.tile_pool(name="const", bufs=1) as cpool,
            tc.tile_pool(name="abuf", bufs=3) as apool,
            tc.tile_pool(name="work", bufs=3) as wpool,
            tc.tile_pool(name="psacc", bufs=2, space="PSUM") as ps_acc,
            tc.tile_pool(name="psz", bufs=2, space="PSUM") as ps_z,
            tc.tile_pool(name="pstr", bufs=2, space="PSUM") as ps_tr,
        ):
            # ---- residents
            tabsb = cpool.tile([P, NB, D], bf16)          # full table (lhsT blocks)
            dis_pt = cpool.tile([P, ntile], f32)
            dis_rep = cpool.tile([D, nshp], bf16)
            zbuf = cpool.tile([D, nshp], bf16)
            act3 = zbuf
            sums = cpool.tile([D, ntile], f32)
            sums2 = cpool.tile([D, ntile], f32)
            W_sb = [cpool.tile([D, D], bf16, tag=f"W{i}", name=f"W{i}_sb")
                    for i in range(3)]
            for i in range(3):
                nc.sync.dma_start(out=W_sb[i][:], in_=Ws[i][:])
            gamma_sb = cpool.tile([D, 1], f32, tag="gamma")
            beta_sb = cpool.tile([D, 1], f32, tag="beta")
            nc.sync.dma_start(out=gamma_sb[:], in_=gamma[:])
            nc.sync.dma_start(out=beta_sb[:], in_=beta[:])
            l1w_sb = cpool.tile([D, D], bf16, tag="l1w")
            l1b_sb = cpool.tile([D, 1], f32, tag="l1b")
            l2w_sb = cpool.tile([D, ncls], bf16, tag="l2w")
            l2b_sb = cpool.tile([ncls, 1], f32, tag="l2b")
            nc.sync.dma_start(out=l1w_sb[:], in_=lin1w[:])
            nc.sync.dma_start(out=l1b_sb[:], in_=lin1b[:])
            nc.sync.dma_start(out=l2w_sb[:], in_=lin2w[:])
            nc.sync.dma_start(out=l2b_sb[:], in_=lin2b[:])
            ident = cpool.tile([D, D], bf16, tag="ident")
            make_identity(nc, ident[:])
            ones1 = cpool.tile([1, D], bf16, tag="ones1")
            nc.gpsimd.memset(ones1[:], 1.0)
            emb = cpool.tile([D, gpc], f32, tag="emb")
            eps_sb = cpool.tile([D, 1], f32, tag="eps")
            nc.gpsimd.memset(eps_sb[:], BN_EPS)

            # zero the table pad rows once (streamed into tabsb; NaN-unsafe)
            if npad > n_nodes:
                zpad = wpool.tile([P, D], bf16, tag="zpad")
                nc.gpsimd.memset(zpad[:], 0.0)
                nc.sync.dma_start(out=table[n_nodes:npad, :],
                                  in_=zpad[:npad - n_nodes, :])

            # ---- dis
            dptf = wpool.tile([P, ntile], f32, tag="dptf")
            nc.sync.dma_start(out=dptf[:], in_=deg_pt[:])
            nc.scalar.activation(dis_pt[:], dptf[:],
                                 mybir.ActivationFunctionType.Sqrt, bias=1.0)
            nc.vector.reciprocal(dis_pt[:], dis_pt[:])
            for o in range(0, nsh, RW):
                w = min(RW, nsh - o)
                dsl = wpool.tile([1, RW], f32, tag="dsl")
                nc.sync.dma_start(out=dsl[:, :w], in_=deg_row[:, o:o + w])
                nc.scalar.activation(dsl[:, :w], dsl[:, :w],
                                     mybir.ActivationFunctionType.Sqrt, bias=1.0)
                nc.vector.reciprocal(dsl[:, :w], dsl[:, :w])
                dslb = wpool.tile([1, RW], bf16, tag="dslb")
                nc.vector.tensor_copy(dslb[:, :w], dsl[:, :w])
                pb = ps_z.tile([D, RW], f32, tag="zt", space="PSUM")
                nc.tensor.matmul(pb[:, :w], lhsT=ones1[:], rhs=dslb[:, :w],
                                 start=True, stop=True)
                nc.vector.tensor_copy(dis_rep[:, o:o + w], pb[:, :w])

            # ---- table0 = bf16(x * dis)
            for t in range(ntile):
                w = tsz[t]
                xt = wpool.tile([P, D], f32, tag="xt")
                nc.sync.dma_start(out=xt[:w, :], in_=x_sh[t * P:t * P + w, :])
                xb = wpool.tile([P, D], bf16, tag="xb")
                nc.scalar.activation(xb[:w, :], xt[:w, :],
                                     mybir.ActivationFunctionType.Copy,
                                     scale=dis_pt[:w, t:t + 1])
                nc.sync.dma_start(out=tshard[t * P:t * P + w, :], in_=xb[:w, :])
            nc.gpsimd.collective_compute(
                "AllGather", mybir.AluOpType.bypass,
                replica_groups=[list(range(N_CORES))],
                ins=[tshard[:, :].opt()], outs=[table[:n_nodes, :].opt()])

            # ---- layers
            for rep in range(reps):
                for li in range(3):
                    last = (li == 2)
                    Wl = W_sb[li]
                    # load the full table as lhsT blocks [s, b, :]
                    # (split: <=32 blocks per DMA keeps descriptor count <=4096)
                    tview = table.rearrange("(b s) d -> s b d", s=P)
                    for b0 in range(0, NB, 32):
                        bw = min(32, NB - b0)
                        nc.sync.dma_start(out=tabsb[:, b0:b0 + bw, :],
                                          in_=tview[:, b0:b0 + bw, :])
                    for r in range(R):
                        rw = rsz[r]
                        ps = ps_acc.tile([D, RW], f32, tag="acc", space="PSUM")
                        for b0 in range(0, NB, BC):
                            bw = min(BC, NB - b0)
                            asb = apool.tile([P, BC, RW], fp8, tag="A")
                            nc.sync.dma_start(
                                out=asb[:, :bw, :],
                                in_=A_in[r, b0:b0 + bw, :, :].rearrange(
                                    "b p d -> p b d"))
                            for bi in range(bw):
                                b = b0 + bi
                                nc.tensor.matmul(
                                    ps[:, :rw], lhsT=tabsb[:, b, :],
                                    rhs=asb[:, bi, :rw],
                                    start=(b == 0), stop=(b == NB - 1))
                        # downstream per node tile in this range
                        for tt in range((rw + P - 1) // P):
                            t = (r * RW) // P + tt
                            w = tsz[t]
                            psl = ps[:, tt * P:tt * P + w]
                            u2t = wpool.tile([D, P], bf16, tag="u2t")
                            nc.vector.tensor_tensor(
                                out=u2t[:, :w], in0=psl,
                                in1=dis_rep[:, t * P:t * P + w],
                                op=mybir.AluOpType.mult)
                            psz = ps_z.tile([D, P], f32, tag="zt", space="PSUM")
                            nc.tensor.matmul(psz[:, :w], lhsT=Wl[:],
                                             rhs=u2t[:, :w],
                                             start=True, stop=True)
                            zslice = (act3 if last else zbuf)[:, t * P:t * P + w]
                            nc.scalar.activation(
                                zslice, psz[:, :w],
                                mybir.ActivationFunctionType.Copy,
                                accum_out=sums[:, t:t + 1])
                            sq = wpool.tile([D, P], f32, tag="sq")
                            nc.scalar.activation(
                                sq[:, :w], psz[:, :w],
                                mybir.ActivationFunctionType.Square,
                                accum_out=sums2[:, t:t + 1])

                    # ---- global BN stats
                    st = wpool.tile([D, 2], f32, tag="st")
                    nc.vector.reduce_sum(st[:, 0:1], sums[:],
                                         axis=mybir.AxisListType.X)
                    nc.vector.reduce_sum(st[:, 1:2], sums2[:],
                                         axis=mybir.AxisListType.X)
                    nc.sync.dma_start(out=stats_in[:], in_=st[:])
                    nc.gpsimd.collective_compute(
                        "AllReduce", mybir.AluOpType.add,
                        replica_groups=[list(range(N_CORES))],
                        ins=[stats_in[:, :].opt()], outs=[stats_out[:, :].opt()])
                    stg = wpool.tile([D, 2], f32, tag="stg")
                    nc.sync.dma_start(out=stg[:], in_=stats_out[:])
                    mu = wpool.tile([D, 1], f32, tag="mu")
                    nc.scalar.activation(mu[:], stg[:, 0:1],
                                         mybir.ActivationFunctionType.Copy,
                                         scale=1.0 / n_nodes)
                    va = wpool.tile([D, 1], f32, tag="va")
                    nc.scalar.activation(va[:], stg[:, 1:2],
                                         mybir.ActivationFunctionType.Copy,
                                         scale=1.0 / n_nodes)
                    mu2 = wpool.tile([D, 1], f32, tag="mu2")
                    nc.vector.tensor_tensor(out=mu2[:], in0=mu[:], in1=mu[:],
                                            op=mybir.AluOpType.mult)
                    nc.vector.tensor_tensor(out=va[:], in0=va[:], in1=mu2[:],
                                            op=mybir.AluOpType.subtract)
                    nc.scalar.activation(va[:], va[:],
                                         mybir.ActivationFunctionType.Sqrt,
                                         bias=eps_sb[:])
                    nc.vector.reciprocal(va[:], va[:])
                    saff = wpool.tile([D, 1], f32, tag="saff")
                    nc.vector.tensor_tensor(out=saff[:], in0=gamma_sb[:],
                                            in1=va[:], op=mybir.AluOpType.mult)
                    tsh_ = wpool.tile([D, 1], f32, tag="tsh")
                    nc.vector.tensor_tensor(out=tsh_[:], in0=mu[:], in1=saff[:],
                                            op=mybir.AluOpType.mult)
                    nc.vector.tensor_tensor(out=tsh_[:], in0=beta_sb[:],
                                            in1=tsh_[:],
                                            op=mybir.AluOpType.subtract)

                    # ---- activation phase
                    for t in range(ntile):
                        w = tsz[t]
                        zsl = (act3 if last else zbuf)[:, t * P:t * P + w]
                        at = wpool.tile([D, P], bf16, tag="at")
                        nc.scalar.activation(at[:, :w], zsl,
                                             mybir.ActivationFunctionType.Relu,
                                             bias=tsh_[:], scale=saff[:])
                        if not last:
                            ht = wpool.tile([D, P], bf16, tag="ht")
                            nc.vector.tensor_tensor(
                                out=ht[:, :w], in0=at[:, :w],
                                in1=dis_rep[:, t * P:t * P + w],
                                op=mybir.AluOpType.mult)
                            ptr = ps_tr.tile([P, D], bf16, tag="tr",
                                             space="PSUM")
                            nc.tensor.transpose(ptr[:w, :], ht[:, :w],
                                                ident[:, :])
                            wr = wpool.tile([P, D], bf16, tag="wr")
                            nc.vector.tensor_copy(wr[:w, :], ptr[:w, :])
                            nc.sync.dma_start(
                                out=tshard[t * P:t * P + w, :], in_=wr[:w, :])
                        else:
                            nc.vector.tensor_copy(act3[:, t * P:t * P + w],
                                                  at[:, :w])
                    if not last:
                        nc.gpsimd.collective_compute(
                            "AllGather", mybir.AluOpType.bypass,
                            replica_groups=[list(range(N_CORES))],
                            ins=[tshard[:, :].opt()],
                            outs=[table[:n_nodes, :].opt()])

            # ---- pooling
            first_seen = set()
            for (t, s0, s1, g) in cfg["pool_segs"]:
                tmp = wpool.tile([D, 1], f32, tag="ptmp")
                nc.vector.reduce_max(tmp[:], act3[:, t * P + s0:t * P + s1],
                                     axis=mybir.AxisListType.X)
                if g not in first_seen:
                    first_seen.add(g)
                    nc.vector.tensor_copy(emb[:, g:g + 1], tmp[:])
                else:
                    nc.vector.tensor_tensor(out=emb[:, g:g + 1],
                                            in0=emb[:, g:g + 1], in1=tmp[:],
                                            op=mybir.AluOpType.max)

            # ---- head
            emb_bf = wpool.tile([D, gpc], bf16, tag="embbf")
            nc.vector.tensor_copy(emb_bf[:], emb[:])
            ph = ps_z.tile([D, gpc], f32, tag="zt", space="PSUM")
            nc.tensor.matmul(ph[:], lhsT=l1w_sb[:], rhs=emb_bf[:],
                             start=True, stop=True)
            h1 = wpool.tile([D, gpc], bf16, tag="h1")
            nc.scalar.activation(h1[:], ph[:],
                                 mybir.ActivationFunctionType.Relu,
                                 bias=l1b_sb[:])
            po = ps_tr.tile([ncls, gpc], f32, tag="tr", space="PSUM")
            nc.tensor.matmul(po[:], lhsT=l2w_sb[:], rhs=h1[:],
                             start=True, stop=True)
            osb = wpool.tile([ncls, gpc], f32, tag="osb")
            nc.scalar.activation(osb[:], po[:],
                                 mybir.ActivationFunctionType.Identity,
                                 bias=l2b_sb[:])
            nc.sync.dma_start(out=out[:, :].rearrange("g c -> c g"), in_=osb[:])

    nc.compile()
    return nc


# ---------------------------------------------------------------- entry point

_CACHE = {}


def _get_built(cfg_key, cfg, reps):
    key = (cfg_key, reps)
    if key not in _CACHE:
        _CACHE[key] = _build(cfg, reps=reps)
    return _CACHE[key]


def kernel(x, edge_index, batch, W1, b1, W2, b2, W3, b3, gamma, beta,
           lin1_w, lin1_b, lin2_w, lin2_b, _reps=1):
    x = np.asarray(x, np.float32)
    edge_index = np.asarray(edge_index)
    batch = np.asarray(batch)
    n_nodes, d = x.shape
    ncls = np.asarray(lin2_w).shape[1]
    assert d == D

    cfg, data = _prep(x, edge_index, batch, ncls)
    nsh = cfg["nsh"]

    # NOTE: b1/b2/b3 cancel inside BatchNorm (mean subtraction) - unused.
    W_bf = [np.asarray(w, np.float32).astype(ml_dtypes.bfloat16)
            for w in (W1, W2, W3)]
    in_maps = []
    for c in range(N_CORES):
        in_maps.append({
            "x_sh": x[c * nsh:(c + 1) * nsh].astype(np.float32),
            "A": data["A"][c],
            "deg_pt": data["deg_pt"][c],
            "deg_row": data["deg_row"][c],
            "W1": W_bf[0], "W2": W_bf[1], "W3": W_bf[2],
            "gamma": np.asarray(gamma, np.float32).reshape(D, 1),
            "beta": np.asarray(beta, np.float32).reshape(D, 1),
            "lin1w": np.asarray(lin1_w, np.float32).astype(ml_dtypes.bfloat16),
            "lin1b": np.asarray(lin1_b, np.float32).reshape(D, 1),
            "lin2w": np.asarray(lin2_w, np.float32).astype(ml_dtypes.bfloat16),
            "lin2b": np.asarray(lin2_b, np.float32).reshape(ncls, 1),
        })

    cfg_key = (n_nodes, edge_index.shape[1], ncls)
    nc = _get_built(cfg_key, cfg, _reps)
    res = run_bass_kernel_spmd(nc, in_maps, core_ids=list(range(N_CORES)))
    outs = [res.results[c]["out"] for c in range(N_CORES)]
    return np.concatenate(outs, axis=0).astype(np.float32)
